# revision 7
# baseline (speedup 1.0000x reference)
import functools
import math
import os
import time

import numpy as np

LAST_HW_EXEC_NS = None

# ---- static anchor configuration (hardcoded from the problem spec) ----
_ANCHORS = (
    dict(stride=32, size=48, scale=[2 ** (1.0 / 3.0), 2 ** (2.0 / 3.0)],
         aspect_ratio=[0.667, 1, 1.5]),
    dict(stride=64, size=96, scale=[2 ** (1.0 / 3.0), 2 ** (2.0 / 3.0)],
         aspect_ratio=[0.667, 1, 1.5]),
    dict(stride=128, size=192, scale=[1, 2 ** (1.0 / 3.0), 2 ** (2.0 / 3.0)],
         aspect_ratio=[0.667, 1, 1.5]),
)


def _anchor_configs():
    cfgs = []
    for info in _ANCHORS:
        stride, size = info['stride'], info['size']
        for scale in info['scale']:
            for ar in info['aspect_ratio']:
                kernel = (int(size * scale / float(ar) ** 0.5),
                          int(size * scale * float(ar) ** 0.5))
                padding = (math.ceil((kernel[0] - stride) / 2.0),
                           math.ceil((kernel[1] - stride) / 2.0))
                cfgs.append((kernel, (stride, stride), padding))
    return cfgs


@functools.lru_cache(maxsize=None)
def _resize_mat(in_size, out_size):
    # jax.image.resize(method='bilinear', antialias=True) weight matrix,
    # shape (in_size, out_size); out[o] = sum_i W[i,o] x[i]
    dt = np.float32
    scale = dt(out_size) / dt(in_size)
    inv_scale = dt(1.0) / scale
    kernel_scale = np.maximum(inv_scale, dt(1.0))
    sample_f = (np.arange(out_size, dtype=dt) + dt(0.5)) * inv_scale - dt(0.5)
    x = np.abs(sample_f[None, :] - np.arange(in_size, dtype=dt)[:, None]) / kernel_scale
    w = np.maximum(dt(0.0), dt(1.0) - np.abs(x))
    tot = w.sum(axis=0, keepdims=True)
    w = np.where(np.abs(tot) > 1000.0 * np.finfo(np.float32).eps,
                 w / np.where(tot != 0, tot, 1), 0)
    ok = np.logical_and(sample_f >= -0.5, sample_f <= in_size - 0.5)
    return np.where(ok[None, :], w, 0).astype(np.float32)


_CFGS = _anchor_configs()
_GHS = [14] * 6 + [7] * 6 + [4] * 9
_NB = {32: 14, 64: 7, 128: 4}

# -------------------------------------------------------------------------
# Trainium path: per-core program (batch element per core), expressed as
# stride-block GEMMs + separable bilinear-resize matmuls; pmap over 8 NCs.
# -------------------------------------------------------------------------
_PMAPPED = None


@functools.lru_cache(maxsize=None)
def _scatter_mat(gh, nb):
    # S[(i,j), (qc*3+qr), (r*nb+c)] = 1 iff r=i+qr-1 in [0,nb) and
    # c=j+qc-1 in [0,nb); gives W3[k,(qc,qr),(r,c)] = w[k,i,j] via matmul.
    S = np.zeros((gh * gh, 9, nb * nb), np.float32)
    for i in range(gh):
        for j in range(gh):
            for qr in range(3):
                r = i + qr - 1
                if not (0 <= r < nb):
                    continue
                for qc in range(3):
                    c = j + qc - 1
                    if 0 <= c < nb:
                        S[i * gh + j, qc * 3 + qr, r * nb + c] = 1.0
    return S.reshape(gh * gh, 9 * nb * nb)


@functools.lru_cache(maxsize=None)
def _embedded_resize(k_sz, s, pad):
    # (3s, 224) resize matrix with the (kh,224) bilinear weights embedded at
    # row offset s-pad; rows outside the window are zero, so no slicing of
    # the 3s-wide block tensor is ever needed.
    Z = np.zeros((3 * s, 224), np.float32)
    Z[s - pad:s - pad + k_sz] = _resize_mat(k_sz, 224)
    return Z


def _percore_fn(jnp):
    def percore(x, wp3, wp4, wp5):
        # x: (448,448,3) f32; wp3: (6,4,14,14); wp4: (6,4,7,7); wp5: (9,4,4,4)
        K = wp3.shape[1]
        X2s = {}
        for s, nb in ((32, 14), (64, 7), (128, 4)):
            side = nb * s
            xp = x if side == 448 else jnp.pad(
                x, ((0, side - 448), (0, side - 448), (0, 0)))
            X2s[s] = xp.reshape(nb, s, nb, s, 3).transpose(
                0, 2, 1, 3, 4).reshape(nb * nb, s * s * 3)
        wlist = ([wp3[a] for a in range(6)] + [wp4[a] for a in range(6)]
                 + [wp5[a] for a in range(9)])
        acc = jnp.zeros((K, 224, 224, 3), jnp.float32)
        for w, ((kh, kw), (s, _), (p0, p1)), gh in zip(wlist, _CFGS, _GHS):
            nb = _NB[s]
            S = jnp.asarray(_scatter_mat(gh, nb))
            Zh = jnp.asarray(_embedded_resize(kh, s, p0))
            Zw = jnp.asarray(_embedded_resize(kw, s, p1))
            W3 = (w.reshape(K, gh * gh) @ S).reshape(K, 9, nb * nb)
            out = jnp.einsum('kqt,tn->kqn', W3, X2s[s],
                             preferred_element_type=jnp.float32)
            out6 = out.reshape(K, 3, 3 * s, s, 3)  # (K, qc, rho, v, t)
            t1 = jnp.einsum('kbrvt,rH->kHtbv', out6, Zh,
                            preferred_element_type=jnp.float32)
            t1f = t1.reshape(K, 224, 3, 3 * s)
            o = jnp.einsum('khts,sW->khWt', t1f, Zw,
                           preferred_element_type=jnp.float32)
            acc = acc + o
        return acc

    return percore


def _get_pmapped():
    global _PMAPPED
    if _PMAPPED is None:
        if os.environ.get('JAX_PLATFORMS') == 'cpu':
            del os.environ['JAX_PLATFORMS']
        import jax
        import jax.numpy as jnp
        try:
            jax.config.update('jax_compilation_cache_dir',
                              '/tmp/jax_nrt_cache')
            jax.config.update('jax_persistent_cache_min_compile_time_secs', 1)
        except Exception:
            pass
        devs = [d for d in jax.devices() if d.platform != 'cpu'][:8]
        if len(devs) < 8:
            raise RuntimeError('need 8 neuron cores, got %d' % len(devs))
        _PMAPPED = (jax.pmap(_percore_fn(jnp), devices=devs), jax, devs)
    return _PMAPPED


def _kernel_trn(x, weights_p3, weights_p4, weights_p5):
    global LAST_HW_EXEC_NS
    f, jax, devs = _get_pmapped()
    B, K = x.shape[0], weights_p3.shape[2]
    shard = lambda a: jax.device_put_sharded([a[i] for i in range(B)], devs)
    args = [shard(a) for a in (x, weights_p3, weights_p4, weights_p5)]
    out = f(*args)  # compile + warm run
    out.block_until_ready()
    best = None
    for _ in range(3):  # best-of-3: the axon RPC jitter dominates variance
        t0 = time.perf_counter()
        out = f(*args)
        out.block_until_ready()
        dt = time.perf_counter() - t0
        best = dt if best is None or dt < best else best
    LAST_HW_EXEC_NS = int(best * 1e9)
    return np.asarray(out).reshape(B * K, 224, 224, 3)


# -------------------------------------------------------------------------
# NumPy fallback (correct, CPU-only) — used only if the neuron path fails.
# -------------------------------------------------------------------------
def _kernel_numpy(x, weights_p3, weights_p4, weights_p5):
    B = x.shape[0]
    K = weights_p3.shape[2]
    weights = ([weights_p3[:, a] for a in range(weights_p3.shape[1])]
               + [weights_p4[:, a] for a in range(weights_p4.shape[1])]
               + [weights_p5[:, a] for a in range(weights_p5.shape[1])])
    acc = np.zeros((B * K, 224, 3, 224), np.float32)
    cur_s, nb, X2 = 0, 0, None
    for w, (kernel, stride, padding) in zip(weights, _CFGS):
        kh, kw = kernel
        s = stride[0]
        p0, p1 = padding
        gh, gw = w.shape[2], w.shape[3]
        if s != cur_s:
            cur_s = s
            nb = -(-x.shape[1] // s)
            if nb * s != x.shape[1]:
                xpad = np.zeros((B, nb * s, nb * s, 3), np.float32)
                xpad[:, :x.shape[1], :x.shape[2]] = x
            else:
                xpad = x
            X2 = np.ascontiguousarray(
                xpad.reshape(B, nb, s, nb, s, 3).transpose(0, 1, 3, 2, 4, 5)
            ).reshape(B, nb * nb, s * s * 3)
        QR = 3
        W3 = np.zeros((B, nb, nb, K, QR, QR), np.float32)
        wtr = np.ascontiguousarray(w, np.float32).transpose(0, 2, 3, 1)
        for qr in range(QR):
            ilo, ihi = max(0, 1 - qr), min(gh, nb + 1 - qr)
            for qc in range(QR):
                jlo, jhi = max(0, 1 - qc), min(gw, nb + 1 - qc)
                W3[:, ilo + qr - 1: ihi + qr - 1, jlo + qc - 1: jhi + qc - 1,
                   :, qr, qc] = wtr[:, ilo:ihi, jlo:jhi]
        out = np.matmul(
            W3.reshape(B, nb * nb, K * QR * QR).transpose(0, 2, 1), X2)
        big = out.reshape(B, K, QR, QR, s, s, 3).transpose(
            0, 1, 2, 4, 3, 5, 6).reshape(B * K, QR * s, QR * s, 3)
        agg = big[:, s - p0: s - p0 + kh, s - p1: s - p1 + kw]
        rh = _resize_mat(kh, 224)
        t1 = np.moveaxis(np.tensordot(rh, agg, axes=([0], [1])), 0, 1)
        rw = _resize_mat(kw, 224)
        acc += np.tensordot(t1, rw, axes=([2], [0]))
    return np.ascontiguousarray(acc.transpose(0, 1, 3, 2))


def kernel(x, weights_p3, weights_p4, weights_p5):
    x = np.asarray(x, np.float32)
    weights_p3 = np.asarray(weights_p3, np.float32)
    weights_p4 = np.asarray(weights_p4, np.float32)
    weights_p5 = np.asarray(weights_p5, np.float32)
    try:
        return _kernel_trn(x, weights_p3, weights_p4, weights_p5)
    except Exception as e:
        import traceback
        traceback.print_exc()
        print('neuron path failed (%r); falling back to numpy' % (e,),
              flush=True)
        return _kernel_numpy(x, weights_p3, weights_p4, weights_p5)
